# revision 15
# baseline (speedup 1.0000x reference)
"""Trainium2 Bass kernel for nn_AttnBlock (GroupNorm + single-head attention
block over [b=4, c=512, l=4096] fp32, 8 NeuronCores).

Sharding: core = (batch, query-half). Each core gets one batch item with its
query half permuted to columns 0..2047 (GroupNorm/attention are invariant to
a consistent permutation of l), computes the full block for its 2048 query
positions, and the host reassembles the [4, 512, 4096] output.

Host passes x twice (bf16 for stats+residual, fp8 for the QKV matmuls) and
the transposed weights in bf16, so the kernel does no on-chip casting of x:
the prologue is DMA-bound at ~8 MB. GroupNorm stats are computed as
sum (DVE reduce) + sum-of-squares (ACT Square with accum), combined across
groups with tiny indicator matmuls, and folded into the QKV weights. All
large matmuls run fp8 DoubleRow. Attention is software-pipelined: the
row-sum + O-accumulation matmuls for step t issue after the S matmuls of
step t+1, and each i-block's epilogue (1/s broadcast, O normalization,
projection, residual add) is interleaved into the first iterations of the
next i-block so the PE never drains. O is normalized by 1/s before the
projection (1/32 folded into wp, 32 into 1/s) so the output epilogue is a
single fused (pj + bp3) + x op per channel block.
"""
import os
import sys
from contextlib import ExitStack

import numpy as np

sys.path.insert(0, "/opt/trn_rl_repo")

import concourse.bass as bass
import concourse.tile as tile
from concourse import bacc, mybir

F32 = mybir.dt.float32
BF16 = mybir.dt.bfloat16
F8 = mybir.dt.float8e4

B, C, L = 4, 512, 4096
NQ = L // 2          # queries per core
P = 128
CO = C // P          # 4 channel blocks
NJT = L // P         # 32 j-tiles
NIB = NQ // 512      # 4 i-blocks
NT = NJT // 2        # 16 t-steps (j-tile pairs) per i-block
NG = 32              # groups
GSZ = C // NG        # 16 channels per group
GPP = P // GSZ       # 8 groups per 128 partitions
EPS = 1e-6
SCALE = float(C) ** -0.5
HC = L // 2          # half-length chunk for x16 streaming

ADD = mybir.AluOpType.add


def build_program():
    nc = bacc.Bacc("TRN2")
    x16_d = nc.declare_dram_parameter("x16", [C, L], BF16, isOutput=False)
    x8_d = nc.declare_dram_parameter("x8", [C, L], F8, isOutput=False)
    wq_d = nc.declare_dram_parameter("wqT", [C, C], BF16, isOutput=False)
    wk_d = nc.declare_dram_parameter("wkT", [C, C], BF16, isOutput=False)
    wv_d = nc.declare_dram_parameter("wvT", [C, C], BF16, isOutput=False)
    wp_d = nc.declare_dram_parameter("wpT", [C, C], BF16, isOutput=False)
    gns_d = nc.declare_dram_parameter("gn_scale", [C], F32, isOutput=False)
    gnb_d = nc.declare_dram_parameter("gn_bias", [C], F32, isOutput=False)
    bq_d = nc.declare_dram_parameter("bq", [C], F32, isOutput=False)
    bv_d = nc.declare_dram_parameter("bv", [C], F32, isOutput=False)
    bp_d = nc.declare_dram_parameter("bp", [C], F32, isOutput=False)
    gm_d = nc.declare_dram_parameter("gmat", [P, GPP], F32, isOutput=False)
    gt_d = nc.declare_dram_parameter("gtmat", [GPP, P], F32, isOutput=False)
    bp3_d = nc.declare_dram_parameter("bp3s", [C], F32, isOutput=True)
    out_d = nc.declare_dram_parameter("out", [C, NQ], F32, isOutput=True)

    with tile.TileContext(nc) as tc:
        attn_block(tc, x16_d, x8_d, wq_d, wk_d, wv_d, wp_d, gns_d, gnb_d,
                   bq_d, bv_d, bp_d, gm_d, gt_d, bp3_d, out_d)
    nc.compile()
    return nc


def attn_block(tc, x16_d, x8_d, wq_d, wk_d, wv_d, wp_d, gns_d, gnb_d,
               bq_d, bv_d, bp_d, gm_d, gt_d, bp3_d, out_d):
    nc = tc.nc
    DR = mybir.MatmulPerfMode.DoubleRow
    x16_v = x16_d.ap().rearrange("(o p) l -> p o l", p=P)
    x8_v = x8_d.ap().rearrange("(r m p) l -> p r m l", p=P, r=2)
    out_v = out_d.ap().rearrange("(o p) i -> p o i", p=P)

    with ExitStack() as ctx:
        big = ctx.enter_context(tc.tile_pool(name="big", bufs=1))
        wbp = ctx.enter_context(tc.tile_pool(name="wbp", bufs=1))
        small = ctx.enter_context(tc.tile_pool(name="small", bufs=1))

        x16_sb = big.tile([P, CO, L], BF16, tag="x16")
        x8_sb = big.tile([P, 2, 2, L], F8, tag="x8")
        q_sb = big.tile([P, 2, 2, NQ], F8, tag="qsb")
        k_sb = big.tile([P, 2, 2, L], F8, tag="ksb")
        vt_sb = big.tile([P, NT, 2, C], F8, tag="vtsb")
        wq_b = wbp.tile([P, 2, 2, C], F8, tag="wqb")
        wk_b = wbp.tile([P, 2, 2, C], F8, tag="wkb")
        wv_b = wbp.tile([P, 2, 2, C], F8, tag="wvb")
        wp_b = wbp.tile([P, 2, 2, C], F8, tag="wpb")

        gns = small.tile([P, CO], F32, tag="gns")
        gnb = small.tile([P, CO], F32, tag="gnb")
        bq_s = small.tile([P, CO], F32, tag="bqs")
        bv_s = small.tile([P, CO], F32, tag="bvs")
        bp_row = small.tile([1, C], F32, tag="bprow")
        bq2 = small.tile([P, CO], F32, tag="bq2")
        bp3pc = small.tile([P, CO], F32, tag="bp3pc")
        ones_p = small.tile([P, 2, 16], F8, tag="onesp")
        ones_1 = small.tile([1, P], F32, tag="ones1")
        nshift = small.tile([P, 1], F32, tag="nshift")

        # ================= prologue: DMA, stats, fold, QKV =================
        with ExitStack() as pctx:
            wf_pool = pctx.enter_context(tc.tile_pool(name="wfp", bufs=1))
            pro = pctx.enter_context(tc.tile_pool(name="pro", bufs=1))
            pro_ps = pctx.enter_context(
                tc.tile_pool(name="prps", bufs=2, space="PSUM"))
            qkv_ps = pctx.enter_context(
                tc.tile_pool(name="qkvps", bufs=5, space="PSUM"))

            # ---- DMA issue order: x16 first (stats gate everything).
            # Bulk data rides the sync (SP) HWDGE ring; the scalar (ACT) ring
            # carries only x16-odd + the 4 weight triggers so the ACT engine
            # queue stays free for the o=3 stats + sqrt chain.
            for o in range(CO):
                for hh in range(2):
                    l0 = hh * HC
                    eng = nc.sync if o % 2 == 0 else nc.scalar
                    eng.dma_start(out=x16_sb[:, o, l0:l0 + HC],
                                  in_=x16_v[:, o, l0:l0 + HC])
            for v_d, v_t in ((gns_d, gns), (gnb_d, gnb), (bq_d, bq_s),
                             (bv_d, bv_s)):
                nc.sync.dma_start(out=v_t[:], in_=v_d.ap().rearrange(
                    "(o p) -> p o", p=P))
            nc.sync.dma_start(out=bp_row[:], in_=bp_d.ap().rearrange(
                "(u c) -> u c", u=1))
            g_mat = pro.tile([P, GPP], F32, tag="gmat")
            nc.sync.dma_start(out=g_mat[:], in_=gm_d.ap())
            gt_mat = pro.tile([GPP, P], F32, tag="gtmat")
            nc.sync.dma_start(out=gt_mat[:], in_=gt_d.ap())
            # Weights + x8 ride the sync ring right behind x16-even, so wk/wq
            # land by ~15us and K is gated only by the stats chain; the
            # scalar ring carries x16-odd + wv/wp.
            wk_f = wf_pool.tile([P, CO, C], BF16, tag="wkf")
            nc.sync.dma_start(out=wk_f[:],
                              in_=wk_d.ap().rearrange("(o p) c -> p o c", p=P))
            wq_f = wf_pool.tile([P, CO, C], BF16, tag="wqf")
            nc.sync.dma_start(out=wq_f[:],
                              in_=wq_d.ap().rearrange("(o p) c -> p o c", p=P))
            wv_f = wf_pool.tile([P, CO, C], BF16, tag="wvf")
            nc.scalar.dma_start(out=wv_f[:],
                                in_=wv_d.ap().rearrange("(o p) c -> p o c", p=P))
            wp_f = wf_pool.tile([P, CO, C], BF16, tag="wpf")
            nc.scalar.dma_start(out=wp_f[:],
                                in_=wp_d.ap().rearrange("(o p) c -> p o c", p=P))
            for hh in range(2):
                l0 = hh * HC
                nc.sync.dma_start(out=x8_sb[:, :, :, l0:l0 + HC],
                                  in_=x8_v[:, :, :, l0:l0 + HC])

            nc.vector.memset(ones_p, 1.0)
            nc.vector.memset(ones_1, 1.0)
            nc.vector.memset(nshift, -3.0)

            # ---- stats: o=0,1 + o2-h0 via DVE bn_stats; o2-h1 + o=3 via ACT
            # Copy/Square accumulators ([P, 2048] ops) — balances the two
            # engines so the stats chain completes ~2us after the x16 DMA.
            bnst = pro.tile([P, 3, 8, 6], F32, tag="bnst")
            for o, hh in ((0, 0), (0, 1), (1, 0), (1, 1), (2, 0)):
                for h in range(HC // 512):
                    l0 = hh * HC + h * 512
                    nc.vector.bn_stats(
                        out=bnst[:, o, hh * (HC // 512) + h, :],
                        in_=x16_sb[:, o, l0:l0 + 512])
            sm3 = pro.tile([P, 2, 3], F32, tag="sm3")
            junk = pro.tile([P, HC], BF16, tag="junk")
            for i, (o, hh) in enumerate(((2, 1), (3, 0), (3, 1))):
                l0 = hh * HC
                nc.scalar.activation(
                    out=junk, in_=x16_sb[:, o, l0:l0 + HC],
                    func=mybir.ActivationFunctionType.Copy,
                    accum_out=sm3[:, 0, i:i + 1])
                nc.scalar.activation(
                    out=junk, in_=x16_sb[:, o, l0:l0 + HC],
                    func=mybir.ActivationFunctionType.Square,
                    accum_out=sm3[:, 1, i:i + 1])
            # dummy Sqrt (scale=0 => sqrt(0), no data dep): pulls the sqrt
            # table load off the critical chain; the dummy Exp after the
            # reciprocal does the same for the exp table.
            nc.scalar.activation(out=junk[:, 0:1], in_=nshift,
                                 func=mybir.ActivationFunctionType.Sqrt,
                                 scale=0.0)
            # st8[:, 0:CO] = per-channel mean, st8[:, CO:] = per-channel E[x^2]
            st8 = pro.tile([P, 2 * CO], F32, tag="st8")
            mv3 = pro.tile([P, 3, 2], F32, tag="mv3")
            nc.vector.bn_aggr(out=mv3[:, 0, :], in_=bnst[:, 0, :, :])
            nc.vector.bn_aggr(out=mv3[:, 1, :], in_=bnst[:, 1, :, :])
            nc.vector.bn_aggr(out=mv3[:, 2, :], in_=bnst[:, 2, 0:4, :])
            # o=0,1 full; o=2 is the h0-half stats only
            nc.vector.tensor_copy(st8[:, 0:3], mv3[:, :, 0])
            nc.vector.tensor_mul(st8[:, CO:CO + 3], mv3[:, :, 0], mv3[:, :, 0])
            nc.vector.tensor_add(st8[:, CO:CO + 3], st8[:, CO:CO + 3],
                                 mv3[:, :, 1])
            # merge: o2 = 0.5*(h0 stats) + (h1 ACT sums)/L;  o3 = sums/L
            sm3s = pro.tile([P, 2, 3], F32, tag="sm3s")
            nc.vector.tensor_scalar_mul(sm3s, sm3, 1.0 / L)
            sm3r = pro.tile([P, 2], F32, tag="sm3r")
            nc.vector.reduce_sum(sm3r[:, 0:1], sm3s[:, 0, 1:3],
                                 mybir.AxisListType.X)
            nc.vector.reduce_sum(sm3r[:, 1:2], sm3s[:, 1, 1:3],
                                 mybir.AxisListType.X)
            nc.vector.scalar_tensor_tensor(
                st8[:, 2:3], st8[:, 2:3], 0.5, sm3s[:, 0, 0:1],
                mybir.AluOpType.mult, ADD)
            nc.vector.tensor_copy(st8[:, 3:4], sm3r[:, 0:1])
            nc.vector.scalar_tensor_tensor(
                st8[:, CO + 2:CO + 3], st8[:, CO + 2:CO + 3], 0.5,
                sm3s[:, 1, 0:1], mybir.AluOpType.mult, ADD)
            nc.vector.tensor_copy(st8[:, CO + 3:CO + 4], sm3r[:, 1:2])

            # ---- group combine via indicator matmuls ----
            gstat_ps = pro_ps.tile([GPP, 2 * CO], F32, tag="mm")
            nc.tensor.matmul(gstat_ps, lhsT=g_mat, rhs=st8, start=True,
                             stop=True)
            mr8 = pro.tile([GPP, 2 * CO], F32, tag="mr8")
            nc.vector.tensor_copy(mr8[:, 0:CO], gstat_ps[:, 0:CO])
            var8 = pro.tile([GPP, CO], F32, tag="var8")
            nc.vector.tensor_mul(var8, mr8[:, 0:CO], mr8[:, 0:CO])
            nc.vector.tensor_sub(var8, gstat_ps[:, CO:2 * CO], var8)
            eps_t = pro.tile([GPP, 1], F32, tag="eps")
            nc.vector.memset(eps_t, EPS)
            sq8 = pro.tile([GPP, CO], F32, tag="sq8")
            nc.scalar.activation(out=sq8, in_=var8,
                                 func=mybir.ActivationFunctionType.Sqrt,
                                 bias=eps_t)
            rscr = pro.tile([GPP, CO], F32, tag="rscr")
            nc.vector.reciprocal_approx_accurate(mr8[:, CO:2 * CO], sq8, rscr)
            # dummy Exp: pull the ACT exp-table load into prologue idle time
            # instead of the QKV->attention seam
            nc.scalar.activation(out=junk[0:GPP, 0:1], in_=sq8[:, 0:1],
                                 func=mybir.ActivationFunctionType.Exp)
            bc_ps = pro_ps.tile([P, 2 * CO], F32, tag="mm")
            nc.tensor.matmul(bc_ps, lhsT=gt_mat, rhs=mr8, start=True,
                             stop=True)
            m44 = pro.tile([P, CO], F32, tag="m44")
            nc.vector.tensor_mul(m44, bc_ps[:, CO:2 * CO], gns)
            a44 = pro.tile([P, CO], F32, tag="a44")
            nc.vector.tensor_mul(a44, bc_ps[:, 0:CO], m44)
            nc.vector.tensor_sub(a44, gnb, a44)
            a44_bf = pro.tile([P, CO], BF16, tag="a44bf")
            nc.vector.tensor_copy(a44_bf, a44)

            def fold(dst, src, scl=None):
                for o in range(CO):
                    s = m44[:, o:o + 1] if scl is None else scl
                    if o % 2 == 0:
                        nc.vector.tensor_scalar_mul(dst[:, o // 2, o % 2, :],
                                                    src[:, o, :], s)
                    else:
                        nc.scalar.activation(
                            out=dst[:, o // 2, o % 2, :], in_=src[:, o, :],
                            func=mybir.ActivationFunctionType.Copy, scale=s)

            # ---- K (weights fold just-in-time before each consumer) ----
            fold(wk_b, wk_f)
            ev = 0
            for lc in range(NJT // 4):
                l0 = lc * 512
                for oc in range(CO):
                    kp = qkv_ps.tile([P, 512], F32, tag="mm")
                    for pr in range(2):
                        nc.tensor.matmul(
                            kp, lhsT=wk_b[:, pr, :, oc * P:(oc + 1) * P],
                            rhs=x8_sb[:, pr, :, l0:l0 + 512],
                            start=(pr == 0), stop=(pr == 1), perf_mode=DR)
                    dst = k_sb[:, oc // 2, oc % 2, l0:l0 + 512]
                    if ev % 2 == 0:
                        nc.vector.tensor_copy(dst, kp)
                    else:
                        nc.scalar.activation(
                            out=dst, in_=kp,
                            func=mybir.ActivationFunctionType.Copy)
                    ev += 1

            # ---- V^T ----
            fold(wv_b, wv_f)
            for jt in range(NJT):
                j0 = jt * P
                vp = qkv_ps.tile([P, C], F32, tag="mm")
                for pr in range(2):
                    nc.tensor.matmul(
                        vp, lhsT=x8_sb[:, pr, :, j0:j0 + P],
                        rhs=wv_b[:, pr, :, :],
                        start=(pr == 0), stop=(pr == 1), perf_mode=DR)
                dst = vt_sb[:, jt // 2, jt % 2, :]
                if ev % 2 == 0:
                    nc.vector.tensor_copy(dst, vp)
                else:
                    nc.scalar.activation(out=dst, in_=vp,
                                         func=mybir.ActivationFunctionType.Copy)
                ev += 1

            # ---- wp fold + bias fixups + bp3, emitted between V and Q so the
            # small matmuls hide inside the evac-bound Q phase instead of
            # delaying the attention start.
            fold(wp_b, wp_f, scl=1.0)
            fold(wq_b, wq_f)
            bv2 = pro.tile([P, CO], F32, tag="bv2")
            for dst_b, w_t, b_t in ((bq2, wq_f, bq_s), (bv2, wv_f, bv_s)):
                for oc in range(CO):
                    mv_ps = pro_ps.tile([P, 1], F32, tag="mm")
                    for cc in range(CO):
                        nc.tensor.matmul(mv_ps,
                                         lhsT=w_t[:, cc, oc * P:(oc + 1) * P],
                                         rhs=a44_bf[:, cc:cc + 1],
                                         start=(cc == 0), stop=(cc == CO - 1))
                    nc.vector.tensor_add(dst_b[:, oc:oc + 1], mv_ps,
                                         b_t[:, oc:oc + 1])
            bv2s = pro.tile([P, CO], F8, tag="bv2s")
            nc.vector.tensor_copy(bv2s, bv2)
            bp3_ps = pro_ps.tile([1, C], F32, tag="mm")
            for cc in range(CO):
                nc.tensor.matmul(bp3_ps, lhsT=bv2s[:, cc:cc + 1],
                                 rhs=wp_b[:, cc // 2, cc % 2, :],
                                 start=(cc == 0), stop=(cc == CO - 1))
            bp3_f = pro.tile([1, C], F32, tag="bp3f")
            nc.vector.tensor_add(bp3_f, bp3_ps, bp_row)
            # roundtrip through DRAM to transpose [1, C] -> [P, CO]
            # (same queue => ordered)
            nc.sync.dma_start(out=bp3_d.ap().rearrange("(u c) -> u c", u=1),
                              in_=bp3_f[:])
            nc.sync.dma_start(out=bp3pc[:],
                              in_=bp3_d.ap().rearrange("(o p) -> p o", p=P))

            # ---- Q ----
            for lc in range(NIB):
                l0 = lc * 512
                for oc in range(CO):
                    qp = qkv_ps.tile([P, 512], F32, tag="mm")
                    for pr in range(2):
                        nc.tensor.matmul(
                            qp, lhsT=wq_b[:, pr, :, oc * P:(oc + 1) * P],
                            rhs=x8_sb[:, pr, :, l0:l0 + 512],
                            start=(pr == 0), stop=(pr == 1), perf_mode=DR)
                    dst = q_sb[:, oc // 2, oc % 2, l0:l0 + 512]
                    if ev % 2 == 0:
                        nc.vector.tensor_scalar_add(dst, qp,
                                                    bq2[:, oc:oc + 1])
                    else:
                        nc.scalar.add(dst, qp, bq2[:, oc:oc + 1])
                    ev += 1

        # ================= attention, software-pipelined =================
        with ExitStack() as actx:
            stp = actx.enter_context(
                tc.tile_pool(name="stp", bufs=2, space="PSUM"))
            op = actx.enter_context(
                tc.tile_pool(name="op", bufs=4, space="PSUM"))
            smp = actx.enter_context(
                tc.tile_pool(name="smp", bufs=2, space="PSUM"))
            p_pool = actx.enter_context(tc.tile_pool(name="ppool", bufs=4))
            osb_pool = actx.enter_context(tc.tile_pool(name="osb", bufs=2))
            out_pool = actx.enter_context(tc.tile_pool(name="outp", bufs=4))
            tinyp = actx.enter_context(tc.tile_pool(name="tiny", bufs=4))
            rbp = actx.enter_context(tc.tile_pool(name="rbp", bufs=2))

            st_state = {}  # ib -> (s_ps, o_ps[cc]..)
            p_tiles = {}   # kk -> p tile
            o_sbs = {}     # ib -> normalized O in fp8

            def emit_s(kk):
                ib, t = divmod(kk, NT)
                i0 = ib * 512
                p_t = p_pool.tile([P, 2, 512], F8, tag="pt")
                p_tiles[kk] = p_t
                for ko in range(2):
                    jt = 2 * t + ko
                    st = stp.tile([P, 512], F32, tag="st")
                    for pr in range(2):
                        nc.tensor.matmul(
                            st, lhsT=k_sb[:, pr, :, jt * P:(jt + 1) * P],
                            rhs=q_sb[:, pr, :, i0:i0 + 512],
                            start=(pr == 0), stop=(pr == 1), perf_mode=DR)
                    nc.scalar.activation(
                        out=p_t[:, ko, :], in_=st,
                        func=mybir.ActivationFunctionType.Exp,
                        bias=nshift, scale=SCALE)

            def emit_sumo(kk):
                ib, t = divmod(kk, NT)
                p_t = p_tiles.pop(kk)
                if t == 0:
                    s_ps = smp.tile([16, 512], F32, tag="sm",
                                    name=f"s{ib}")
                    o_ps = [op.tile([P, 512], F32, tag="oacc",
                                    name=f"o{ib}_{cc}") for cc in range(CO)]
                    st_state[ib] = (s_ps, o_ps)
                s_ps, o_ps = st_state[ib]
                nc.tensor.matmul(s_ps, lhsT=ones_p, rhs=p_t,
                                 start=(t == 0), stop=(t == NT - 1),
                                 perf_mode=DR)
                for cc in range(CO):
                    nc.tensor.matmul(
                        o_ps[cc], lhsT=vt_sb[:, t, :, cc * P:(cc + 1) * P],
                        rhs=p_t, start=(t == 0), stop=(t == NT - 1),
                        perf_mode=DR)

            def emit_epi_a(ib):
                # s -> 1/s (x32) -> broadcast -> normalize O to fp8
                s_ps, o_ps = st_state[ib]
                s_f = tinyp.tile([1, 512], F32, tag="sf")
                nc.vector.tensor_copy(s_f, s_ps[0:1, :])
                rinv = tinyp.tile([1, 512], F32, tag="rinv")
                nc.vector.reciprocal_approx_fast(rinv, s_f)
                rb_ps = smp.tile([P, 512], F32, tag="sm", name=f"rb{ib}")
                nc.tensor.matmul(rb_ps, lhsT=ones_1, rhs=rinv,
                                 start=True, stop=True)
                rinv_b = rbp.tile([P, 512], F32, tag="rinvb")
                nc.vector.tensor_copy(rinv_b, rb_ps)
                o_sb = osb_pool.tile([P, 2, 2, 512], F8, tag="osb")
                for cc in range(CO):
                    nc.vector.tensor_mul(o_sb[:, cc // 2, cc % 2, :],
                                         o_ps[cc], rinv_b)
                o_sbs[ib] = o_sb
                del st_state[ib]

            def emit_proj(ib, ocs):
                i0 = ib * 512
                o_sb = o_sbs[ib]
                for oc in ocs:
                    pj = smp.tile([P, 512], F32, tag="sm", name=f"pj{ib}_{oc}")
                    for pr in range(2):
                        nc.tensor.matmul(
                            pj, lhsT=wp_b[:, pr, :, oc * P:(oc + 1) * P],
                            rhs=o_sb[:, pr, :, :],
                            start=(pr == 0), stop=(pr == 1), perf_mode=DR)
                    out_t = out_pool.tile([P, 512], F32, tag="outt")
                    nc.vector.scalar_tensor_tensor(
                        out_t, pj, bp3pc[:, oc:oc + 1],
                        x16_sb[:, oc, i0:i0 + 512], ADD, ADD)
                    nc.sync.dma_start(out=out_v[:, oc, i0:i0 + 512],
                                      in_=out_t)

            for kk in range(NIB * NT):
                ib, t = divmod(kk, NT)
                emit_s(kk)
                if kk > 0:
                    emit_sumo(kk - 1)
                if ib > 0:
                    if t == 0:
                        emit_epi_a(ib - 1)
                    elif t == 1:
                        emit_proj(ib - 1, (0, 1))
                    elif t == 2:
                        emit_proj(ib - 1, (2, 3))
            # flush: final block's epilogue with o_norm / projection halves
            # interleaved so the PE drains as late as possible
            emit_sumo(NIB * NT - 1)
            ib = NIB - 1
            i0 = ib * 512
            s_ps, o_ps = st_state[ib]
            s_f = tinyp.tile([1, 512], F32, tag="sf")
            nc.vector.tensor_copy(s_f, s_ps[0:1, :])
            rinv = tinyp.tile([1, 512], F32, tag="rinv")
            nc.vector.reciprocal_approx_fast(rinv, s_f)
            rb_ps = smp.tile([P, 512], F32, tag="sm", name="rbF")
            nc.tensor.matmul(rb_ps, lhsT=ones_1, rhs=rinv,
                             start=True, stop=True)
            rinv_b = rbp.tile([P, 512], F32, tag="rinvb")
            nc.vector.tensor_copy(rinv_b, rb_ps)
            o_sb = osb_pool.tile([P, 2, 2, 512], F8, tag="osb")
            for cc in (0, 1):
                nc.vector.tensor_mul(o_sb[:, 0, cc, :], o_ps[cc], rinv_b)
            pjs = []
            for oc in range(CO):
                pj = op.tile([P, 512], F32, tag="oacc", name=f"fpj{oc}")
                nc.tensor.matmul(pj, lhsT=wp_b[:, 0, :, oc * P:(oc + 1) * P],
                                 rhs=o_sb[:, 0, :, :], start=True, stop=False,
                                 perf_mode=DR)
                pjs.append(pj)
            for cc in (2, 3):
                nc.vector.tensor_mul(o_sb[:, 1, cc - 2, :], o_ps[cc], rinv_b)
            for oc in range(CO):
                nc.tensor.matmul(pjs[oc],
                                 lhsT=wp_b[:, 1, :, oc * P:(oc + 1) * P],
                                 rhs=o_sb[:, 1, :, :], start=False, stop=True,
                                 perf_mode=DR)
                out_t = out_pool.tile([P, 512], F32, tag="outt")
                nc.vector.scalar_tensor_tensor(
                    out_t, pjs[oc], bp3pc[:, oc:oc + 1],
                    x16_sb[:, oc, i0:i0 + 512], ADD, ADD)
                nc.sync.dma_start(out=out_v[:, oc, i0:i0 + 512], in_=out_t)


def kernel(**inputs):
    import ml_dtypes

    x = np.ascontiguousarray(np.asarray(inputs["x"], np.float32))
    args = {
        "wqT": np.ascontiguousarray(
            np.asarray(inputs["wq"], np.float32).T).astype(ml_dtypes.bfloat16),
        "wkT": np.ascontiguousarray(
            np.asarray(inputs["wk"], np.float32).T).astype(ml_dtypes.bfloat16),
        "wvT": np.ascontiguousarray(
            np.asarray(inputs["wv"], np.float32).T).astype(ml_dtypes.bfloat16),
        "wpT": np.ascontiguousarray(
            np.asarray(inputs["wp"], np.float32).T).astype(ml_dtypes.bfloat16),
        "gn_scale": np.asarray(inputs["gn_scale"], np.float32),
        "gn_bias": np.asarray(inputs["gn_bias"], np.float32),
        "bq": np.asarray(inputs["bq"], np.float32),
        "bv": np.asarray(inputs["bv"], np.float32),
        "bp": np.asarray(inputs["bp"], np.float32),
    }
    pidx = np.arange(P)
    gmat = (pidx[:, None] // GSZ == np.arange(GPP)[None, :]).astype(np.float32)
    args["gmat"] = np.ascontiguousarray(gmat / float(GSZ))
    args["gtmat"] = np.ascontiguousarray(gmat.T)
    in_maps = []
    for core in range(8):
        bi, half = core // 2, core % 2
        sl = slice(half * NQ, (half + 1) * NQ)
        other = slice((1 - half) * NQ, (2 - half) * NQ)
        xp = np.ascontiguousarray(
            np.concatenate([x[bi][:, sl], x[bi][:, other]], axis=1))
        in_maps.append({
            "x16": xp.astype(ml_dtypes.bfloat16),
            "x8": xp.astype(ml_dtypes.float8_e4m3),
            **args,
        })

    from concourse.bass_utils import run_bass_kernel_spmd

    nc = build_program()
    trace = bool(int(os.environ.get("KERNEL_TRACE", "0")))
    res = run_bass_kernel_spmd(nc, in_maps, core_ids=list(range(8)),
                               trace=trace)
    kernel.last_results = res
    out = np.empty((B, C, L), np.float32)
    for core in range(8):
        bi, half = core // 2, core % 2
        out[bi][:, half * NQ : (half + 1) * NQ] = res.results[core]["out"]
    return out
